# revision 12
# baseline (speedup 1.0000x reference)
"""EngramGating Trainium2 Bass kernel.

Reference computation (per token t, head h, DIM=32, HC_MULT=4):
    key[t,h,:]  = emb[t,:] @ Wk[h].T + bk[h]                  # [4,32]
    nk = key * rsqrt(mean_k(key^2)+eps) * g1
    nq = hid  * rsqrt(mean_k(hid^2)+eps) * g2
    gate0[t,h] = sum_k nk*nq / sqrt(32)
    ga = sign(gate0)*sqrt(max(|gate0|,1e-6));  gate = sigmoid(ga)
    out[t,h,:] = gate[t,h] * (emb[t,:] @ Wv.T + bv)

Sharding: pure data parallel over 8 cores, contiguous token ranges.

Per-core layout: tokens-on-partitions. Each block covers 2304 tokens
(18 tokens per partition = 6 chunks x 3 tokens). emb chunks [128,96]
are PE-transposed, then per chunk 3 row-tiled K=32 matmuls (tokens
j=0..2) + one bias matmul (ones rows x bias row) produce key|value
(with biases) in PSUM. ACT evacuates PSUM->SBUF and squares; DVE does
the three segmented reductions (sum_k key^2, hid^2, key*hid); GPSIMD
does the elementwise products. The scalar tail (sqrt/sign/sigmoid) is
batched per superblock of 5 blocks to amortize ACT table-set loads.
"""

import math
import numpy as np
from contextlib import ExitStack

import concourse.bass as bass
import concourse.bacc as bacc
import concourse.mybir as mybir
import concourse.tile as tile
from concourse.bass_utils import run_bass_kernel_spmd

F32 = mybir.dt.float32
AF = mybir.ActivationFunctionType
ALU = mybir.AluOpType
AX = mybir.AxisListType

# problem dims
B, S, DIM, H = 16, 16384, 32, 4
TOK = B * S                  # 262144
NCORES = 8
TPC = TOK // NCORES          # 32768 tokens per core
HK = H * DIM                 # 128

# block geometry
TPP = 18                     # tokens per partition per block (6 chunks x 3)
BLK = 128 * TPP              # 2304 tokens per block
NCHUNK = 6                   # chunks per block (3 tokens each per partition)
NPAIR = 3                    # chunk pairs
SB_BLKS = 5                  # blocks per superblock (scalar-tail batch)
EPS = float(np.finfo(np.float32).eps)

# block start tokens: 14 full blocks + 1 overlapping tail block
_t0s = [i * BLK for i in range(TPC // BLK)]
if TPC % BLK:
    _t0s.append(TPC - BLK)
T0S = _t0s
NBLK = len(T0S)              # 15
assert NBLK % SB_BLKS == 0


def _build_nc(apply_g12: bool):
    nc = bacc.Bacc(None, target_bir_lowering=False, debug=False)

    emb_d = nc.dram_tensor("emb", [TPC * DIM], F32, kind="ExternalInput")
    hid_d = nc.dram_tensor("hid", [TPC * HK], F32, kind="ExternalInput")
    wkv_d = nc.dram_tensor("wkv", [128, 480], F32, kind="ExternalInput")
    ident_d = nc.dram_tensor("ident", [128, 128], F32, kind="ExternalInput")
    g12_d = None
    if apply_g12:
        g12_d = nc.dram_tensor("g12", [128, HK], F32, kind="ExternalInput")
    out_d = nc.dram_tensor("out", [TPC * HK], F32, kind="ExternalOutput")

    with tile.TileContext(nc) as tc, ExitStack() as ctx:
        const_p = ctx.enter_context(tc.tile_pool(name="const", bufs=1))
        emb_p = ctx.enter_context(tc.tile_pool(name="embp", bufs=2))
        hid_p = ctx.enter_context(tc.tile_pool(name="hidp", bufs=2))
        tp_p = ctx.enter_context(
            tc.tile_pool(name="tpp", bufs=2, space=bass.MemorySpace.PSUM))
        kvp_p = ctx.enter_context(
            tc.tile_pool(name="kvpp", bufs=2, space=bass.MemorySpace.PSUM))
        kvsb_p = ctx.enter_context(tc.tile_pool(name="kvsbp", bufs=2))
        sq_p = ctx.enter_context(tc.tile_pool(name="sqp", bufs=2))
        prod_p = ctx.enter_context(tc.tile_pool(name="prodp", bufs=2))
        stage_p = ctx.enter_context(tc.tile_pool(name="stagep", bufs=2))
        tail_p = ctx.enter_context(tc.tile_pool(name="tailp", bufs=1))
        out_p = ctx.enter_context(tc.tile_pool(name="outp", bufs=2))

        wkv_sb = const_p.tile([128, 480], F32)
        ident_sb = const_p.tile([128, 128], F32)
        eps_k = const_p.tile([128, 1], F32)
        eps_q = const_p.tile([128, 1], F32)
        nc.gpsimd.memset(eps_k[:], 32.0 * EPS)
        nc.gpsimd.memset(eps_q[:], EPS)
        nc.sync.dma_start(wkv_sb[:], wkv_d[:])
        nc.sync.dma_start(ident_sb[:], ident_d[:])
        if apply_g12:
            g12_sb = const_p.tile([128, HK], F32)
            nc.sync.dma_start(g12_sb[:], g12_d[:])

        # persistent embT tiles: rows 96:128 stay all-ones (bias rows for
        # the K=128 block-diagonal matmul); rows 0:96 rewritten per pair.
        embT_tiles = []
        for i in range(3):
            t = const_p.tile([128, 2, 128], F32, name=f"embT{i}")
            nc.gpsimd.memset(t[96:128, :, :], 1.0)
            embT_tiles.append(t)

        nsb = NBLK // SB_BLKS
        for sb in range(nsb):
            # superblock staging
            msk_st = stage_p.tile([128, SB_BLKS, TPP, H], F32, name="msk_st")
            msq_st = stage_p.tile([128, SB_BLKS, TPP, H], F32, name="msq_st")
            dot_st = stage_p.tile([128, SB_BLKS, TPP, H], F32, name="dot_st")
            val_st = stage_p.tile([128, SB_BLKS, TPP, DIM], F32, name="val_st")

            for bb in range(SB_BLKS):
                b = sb * SB_BLKS + bb
                t0 = T0S[b]

                emb_sb = emb_p.tile([128, TPP * DIM], F32, name="emb_sb")
                nc.sync.dma_start(
                    emb_sb[:],
                    emb_d[t0 * DIM:(t0 + BLK) * DIM].rearrange(
                        "(p f) -> p f", p=128))
                hid_sb = hid_p.tile([128, TPP * HK], F32, name="hid_sb")
                nc.sync.dma_start(
                    hid_sb[:],
                    hid_d[t0 * HK:(t0 + BLK) * HK].rearrange(
                        "(p f) -> p f", p=128))

                kv_sb = kvsb_p.tile([128, TPP, 160], F32, name="kv_sb")

                for g in range(NPAIR):
                    tp = tp_p.tile([96, 2, 128], F32, name="tp")
                    for c2 in range(2):
                        cc = 2 * g + c2
                        nc.tensor.matmul(
                            tp[:, c2, :],
                            emb_sb[:, 96 * cc:96 * (cc + 1)],
                            ident_sb[:],
                            is_transpose=True,
                            start=(c2 == 0), stop=(c2 == 1))
                    embT = embT_tiles[g]
                    nc.scalar.copy(embT[0:96, :, :], tp[:])

                    kvp = kvp_p.tile([128, 2, 512], F32, name="kvp")
                    for c2 in range(2):
                        # single K=128 matmul: rows 0:96 = 3 transposed
                        # token-groups against block-diagonal W, rows
                        # 96:128 = ones against the bias row.
                        nc.tensor.matmul(
                            kvp[:, c2, 0:480],
                            embT[:, c2, :],
                            wkv_sb[:, 0:480],
                            start=True, stop=True)
                    # evacuate PSUM -> SBUF (ACT)
                    nc.scalar.copy(
                        kv_sb[:, 6 * g:6 * (g + 1), :].rearrange(
                            "p (c j) m -> p c j m", c=2),
                        kvp[:, :, 0:480].rearrange(
                            "p c (j m) -> p c j m", m=160))

                key4 = kv_sb[:, :, 0:HK].rearrange("p s (h k) -> p s h k", h=H)
                val3 = kv_sb[:, :, HK:160]
                hid4 = hid_sb.rearrange("p (s h k) -> p s h k", s=TPP, h=H)

                sqk = sq_p.tile([128, TPP, H, DIM], F32, name="sqk")
                nc.scalar.activation(sqk[:], key4, AF.Square)
                sqq = sq_p.tile([128, TPP, H, DIM], F32, name="sqq")
                nc.scalar.activation(sqq[:], hid4, AF.Square)

                if apply_g12:
                    prod_in1 = prod_p.tile([128, TPP, H, DIM], F32, name="hidg")
                    nc.vector.tensor_tensor(
                        prod_in1[:], hid4,
                        g12_sb[:].rearrange("p (o h k) -> p o h k", o=1, h=H)
                        .broadcast_to([128, TPP, H, DIM]),
                        op=ALU.mult)
                    prod_in1 = prod_in1[:]
                else:
                    prod_in1 = hid4

                prod = prod_p.tile([128, TPP, H, DIM], F32, name="prod")
                nc.gpsimd.tensor_tensor(prod[:], key4, prod_in1, op=ALU.mult)

                nc.vector.reduce_sum(msk_st[:, bb, :, :], sqk[:], axis=AX.X)
                nc.vector.reduce_sum(msq_st[:, bb, :, :], sqq[:], axis=AX.X)
                nc.vector.reduce_sum(dot_st[:, bb, :, :], prod[:], axis=AX.X)
                nc.scalar.copy(val_st[:, bb, :, :], val3)

            # ---- superblock scalar tail ----
            FT = SB_BLKS * TPP * H
            sk = tail_p.tile([128, FT], F32, name="sk")
            msk_f = msk_st[:].rearrange("p a b c -> p (a b c)")
            msq_f = msq_st[:].rearrange("p a b c -> p (a b c)")
            dot_f = dot_st[:].rearrange("p a b c -> p (a b c)")
            # sk = sqrt(sum_k key^2 + 32*eps) = sqrt(32)*sqrt(mean+eps)
            nc.scalar.activation(sk[:], msk_f, AF.Sqrt, bias=eps_k[:])
            sq2 = tail_p.tile([128, FT], F32, name="sq2")
            nc.scalar.activation(sq2[:], msq_f, AF.Sqrt,
                                 bias=eps_q[:], scale=1.0 / 32.0)
            sg = tail_p.tile([128, FT], F32, name="sg")
            nc.scalar.activation(sg[:], dot_f, AF.Sign)
            den = tail_p.tile([128, FT], F32, name="den")
            nc.vector.tensor_tensor(den[:], sk[:], sq2[:], op=ALU.mult)
            rden = tail_p.tile([128, FT], F32, name="rden")
            nc.vector.reciprocal(rden[:], den[:])
            g0 = tail_p.tile([128, FT], F32, name="g0")
            nc.vector.tensor_tensor(g0[:], dot_f, rden[:], op=ALU.mult)
            aa = tail_p.tile([128, FT], F32, name="aa")
            nc.scalar.activation(aa[:], g0[:], AF.Abs)
            m = tail_p.tile([128, FT], F32, name="m")
            nc.vector.tensor_scalar(m[:], aa[:], 1e-6, None, op0=ALU.max)
            r = tail_p.tile([128, FT], F32, name="r")
            nc.scalar.activation(r[:], m[:], AF.Sqrt)
            rs = tail_p.tile([128, FT], F32, name="rs")
            nc.vector.tensor_tensor(rs[:], r[:], sg[:], op=ALU.mult)
            gate = tail_p.tile([128, SB_BLKS, TPP, H], F32, name="gate")
            nc.scalar.activation(
                gate[:].rearrange("p a b c -> p (a b c)"), rs[:], AF.Sigmoid)

            # ---- final gating + store ----
            for bb in range(SB_BLKS):
                b = sb * SB_BLKS + bb
                t0 = T0S[b]
                out_sb = out_p.tile([128, TPP, H, DIM], F32, name="out_sb")
                gate_b = gate[:, bb, :, :].unsqueeze(3)
                val_b = val_st[:, bb, :, :].unsqueeze(2)
                # split final elementwise mul between DVE (3/18) and GPSIMD
                SPL = 3
                nc.vector.tensor_tensor(
                    out_sb[:, 0:SPL, :, :],
                    gate_b[:, 0:SPL, :, :].broadcast_to([128, SPL, H, DIM]),
                    val_b[:, 0:SPL, :, :].broadcast_to([128, SPL, H, DIM]),
                    op=ALU.mult)
                nc.gpsimd.tensor_tensor(
                    out_sb[:, SPL:TPP, :, :],
                    gate_b[:, SPL:TPP, :, :].broadcast_to(
                        [128, TPP - SPL, H, DIM]),
                    val_b[:, SPL:TPP, :, :].broadcast_to(
                        [128, TPP - SPL, H, DIM]),
                    op=ALU.mult)
                nc.sync.dma_start(
                    out_d[t0 * HK:(t0 + BLK) * HK].rearrange(
                        "(p f) -> p f", p=128),
                    out_sb[:].rearrange("p a b c -> p (a b c)"))

    nc.compile()
    return nc


def _prep_consts(Wv, bv, Wk, bk):
    # Wkv_cat[d, h*32+k] = Wk[h,k,d];  Wkv_cat[d, 128+v] = Wv[v,d]
    wkv_cat = np.zeros((DIM, 160), dtype=np.float32)
    wkv_cat[:, 0:HK] = np.transpose(Wk, (2, 0, 1)).reshape(DIM, HK)
    wkv_cat[:, HK:160] = Wv.T
    bias_cat = np.concatenate(
        [bk.reshape(HK).astype(np.float32), bv.astype(np.float32)])
    wkv = np.zeros((128, 480), dtype=np.float32)
    for j in range(3):
        wkv[32 * j:32 * (j + 1), 160 * j:160 * (j + 1)] = wkv_cat
    wkv[96, :] = np.tile(bias_cat, 3)
    ident = np.eye(128, dtype=np.float32)
    return wkv, ident


_CACHE = {}


def kernel_with_results(embeddings, hidden_states, Wv, bv, Wk, bk, g1, g2,
                        **run_kwargs):
    embeddings = np.ascontiguousarray(np.asarray(embeddings, dtype=np.float32))
    hidden_states = np.ascontiguousarray(
        np.asarray(hidden_states, dtype=np.float32))
    Wv = np.asarray(Wv, dtype=np.float32)
    bv = np.asarray(bv, dtype=np.float32)
    Wk = np.asarray(Wk, dtype=np.float32)
    bk = np.asarray(bk, dtype=np.float32)
    g12 = (np.asarray(g1, np.float32) * np.asarray(g2, np.float32))
    apply_g12 = not np.all(g12 == 1.0)

    if apply_g12 not in _CACHE:
        _CACHE[apply_g12] = _build_nc(apply_g12)
    nc = _CACHE[apply_g12]

    wkv, ident = _prep_consts(Wv, bv, Wk, bk)

    emb_flat = embeddings.reshape(TOK, DIM)
    hid_flat = hidden_states.reshape(TOK, HK)

    in_maps = []
    for c in range(NCORES):
        m = {
            "emb": np.ascontiguousarray(
                emb_flat[c * TPC:(c + 1) * TPC]).reshape(-1),
            "hid": np.ascontiguousarray(
                hid_flat[c * TPC:(c + 1) * TPC]).reshape(-1),
            "wkv": wkv,
            "ident": ident,
        }
        if apply_g12:
            m["g12"] = np.tile(
                g12.reshape(1, HK), (128, 1)).astype(np.float32)
        in_maps.append(m)

    res = run_bass_kernel_spmd(nc, in_maps, core_ids=list(range(NCORES)),
                               **run_kwargs)
    out = np.concatenate(
        [res.results[c]["out"].reshape(TPC, HK) for c in range(NCORES)],
        axis=0)
    return out.reshape(B, S, H, DIM), res


def kernel(embeddings, hidden_states, Wv, bv, Wk, bk, g1, g2):
    out, _ = kernel_with_results(
        embeddings, hidden_states, Wv, bv, Wk, bk, g1, g2)
    return out


# revision 14
# speedup vs baseline: 9.8127x; 9.8127x over previous
"""EngramGating Trainium2 Bass kernel.

Reference computation (per token t, head h, DIM=32, HC_MULT=4):
    key[t,h,:]  = emb[t,:] @ Wk[h].T + bk[h]                  # [4,32]
    nk = key * rsqrt(mean_k(key^2)+eps) * g1
    nq = hid  * rsqrt(mean_k(hid^2)+eps) * g2
    gate0[t,h] = sum_k nk*nq / sqrt(32)
    ga = sign(gate0)*sqrt(max(|gate0|,1e-6));  gate = sigmoid(ga)
    out[t,h,:] = gate[t,h] * (emb[t,:] @ Wv.T + bv)

Sharding: pure data parallel over 8 cores, contiguous token ranges.

Per-core layout: tokens-on-partitions. Each block covers 2304 tokens
(18 tokens per partition = 6 chunks x 3 tokens). emb chunks [128,96]
are PE-transposed, then per chunk 3 row-tiled K=32 matmuls (tokens
j=0..2) + one bias matmul (ones rows x bias row) produce key|value
(with biases) in PSUM. ACT evacuates PSUM->SBUF and squares; DVE does
the three segmented reductions (sum_k key^2, hid^2, key*hid); GPSIMD
does the elementwise products. The scalar tail (sqrt/sign/sigmoid) is
batched per superblock of 5 blocks to amortize ACT table-set loads.
"""

import math
import numpy as np
from contextlib import ExitStack

import concourse.bass as bass
import concourse.bacc as bacc
import concourse.mybir as mybir
import concourse.tile as tile
from concourse.bass_utils import run_bass_kernel_spmd

F32 = mybir.dt.float32
AF = mybir.ActivationFunctionType
ALU = mybir.AluOpType
AX = mybir.AxisListType

# problem dims
B, S, DIM, H = 16, 16384, 32, 4
TOK = B * S                  # 262144
NCORES = 8
TPC = TOK // NCORES          # 32768 tokens per core
HK = H * DIM                 # 128

# block geometry
TPP = 18                     # tokens per partition per block (6 chunks x 3)
BLK = 128 * TPP              # 2304 tokens per block
NCHUNK = 6                   # chunks per block (3 tokens each per partition)
NPAIR = 3                    # chunk pairs
SB_BLKS = 5                  # blocks per superblock (scalar-tail batch)
EPS = float(np.finfo(np.float32).eps)

# block start tokens: 14 full blocks + 1 overlapping tail block
_t0s = [i * BLK for i in range(TPC // BLK)]
if TPC % BLK:
    _t0s.append(TPC - BLK)
T0S = _t0s
NBLK = len(T0S)              # 15
assert NBLK % SB_BLKS == 0


def _build_nc(apply_g12: bool, reps: int = 1):
    nc = bacc.Bacc(None, target_bir_lowering=False, debug=False)

    emb_d = nc.dram_tensor("emb", [TPC * DIM], F32, kind="ExternalInput")
    hid_d = nc.dram_tensor("hid", [TPC * HK], F32, kind="ExternalInput")
    wkv_d = nc.dram_tensor("wkv", [128, 480], F32, kind="ExternalInput")
    ident_d = nc.dram_tensor("ident", [128, 128], F32, kind="ExternalInput")
    g12_d = None
    if apply_g12:
        g12_d = nc.dram_tensor("g12", [128, HK], F32, kind="ExternalInput")
    out_d = nc.dram_tensor("out", [TPC * HK], F32, kind="ExternalOutput")

    with tile.TileContext(nc) as tc, ExitStack() as ctx:
        const_p = ctx.enter_context(tc.tile_pool(name="const", bufs=1))
        emb_p = ctx.enter_context(tc.tile_pool(name="embp", bufs=2))
        hid_p = ctx.enter_context(tc.tile_pool(name="hidp", bufs=2))
        tp_p = ctx.enter_context(
            tc.tile_pool(name="tpp", bufs=2, space=bass.MemorySpace.PSUM))
        kvp_p = ctx.enter_context(
            tc.tile_pool(name="kvpp", bufs=2, space=bass.MemorySpace.PSUM))
        kvsb_p = ctx.enter_context(tc.tile_pool(name="kvsbp", bufs=2))
        sq_p = ctx.enter_context(tc.tile_pool(name="sqp", bufs=2))
        prod_p = ctx.enter_context(tc.tile_pool(name="prodp", bufs=2))
        stage_p = ctx.enter_context(tc.tile_pool(name="stagep", bufs=2))
        tail_p = ctx.enter_context(tc.tile_pool(name="tailp", bufs=1))
        out_p = ctx.enter_context(tc.tile_pool(name="outp", bufs=2))

        wkv_sb = const_p.tile([128, 480], F32)
        ident_sb = const_p.tile([128, 128], F32)
        eps_k = const_p.tile([128, 1], F32)
        eps_q = const_p.tile([128, 1], F32)
        nc.gpsimd.memset(eps_k[:], 32.0 * EPS)
        nc.gpsimd.memset(eps_q[:], EPS)
        nc.sync.dma_start(wkv_sb[:], wkv_d[:])
        nc.sync.dma_start(ident_sb[:], ident_d[:])
        if apply_g12:
            g12_sb = const_p.tile([128, HK], F32)
            nc.sync.dma_start(g12_sb[:], g12_d[:])

        # persistent embT tiles: rows 96:128 stay all-ones (bias rows for
        # the K=128 block-diagonal matmul); rows 0:96 rewritten per pair.
        embT_tiles = []
        for i in range(3):
            t = const_p.tile([128, 2, 128], F32, name=f"embT{i}")
            nc.gpsimd.memset(t[96:128, :, :], 1.0)
            embT_tiles.append(t)

        nsb = NBLK // SB_BLKS
        for sb in [s for _ in range(reps) for s in range(nsb)]:
            # superblock staging
            msk_st = stage_p.tile([128, SB_BLKS, TPP, H], F32, name="msk_st")
            msq_st = stage_p.tile([128, SB_BLKS, TPP, H], F32, name="msq_st")
            dot_st = stage_p.tile([128, SB_BLKS, TPP, H], F32, name="dot_st")
            val_st = stage_p.tile([128, SB_BLKS, TPP, DIM], F32, name="val_st")

            for bb in range(SB_BLKS):
                b = sb * SB_BLKS + bb
                t0 = T0S[b]

                emb_sb = emb_p.tile([128, TPP * DIM], F32, name="emb_sb")
                nc.sync.dma_start(
                    emb_sb[:],
                    emb_d[t0 * DIM:(t0 + BLK) * DIM].rearrange(
                        "(p f) -> p f", p=128))
                hid_sb = hid_p.tile([128, TPP * HK], F32, name="hid_sb")
                nc.sync.dma_start(
                    hid_sb[:],
                    hid_d[t0 * HK:(t0 + BLK) * HK].rearrange(
                        "(p f) -> p f", p=128))

                kv_sb = kvsb_p.tile([128, TPP, 160], F32, name="kv_sb")

                for g in range(NPAIR):
                    tp = tp_p.tile([96, 2, 128], F32, name="tp")
                    for c2 in range(2):
                        cc = 2 * g + c2
                        nc.tensor.matmul(
                            tp[:, c2, :],
                            emb_sb[:, 96 * cc:96 * (cc + 1)],
                            ident_sb[:],
                            is_transpose=True,
                            start=(c2 == 0), stop=(c2 == 1))
                    embT = embT_tiles[g]
                    nc.scalar.copy(embT[0:96, :, :], tp[:])

                    kvp = kvp_p.tile([128, 2, 512], F32, name="kvp")
                    for c2 in range(2):
                        # single K=128 matmul: rows 0:96 = 3 transposed
                        # token-groups against block-diagonal W, rows
                        # 96:128 = ones against the bias row.
                        nc.tensor.matmul(
                            kvp[:, c2, 0:480],
                            embT[:, c2, :],
                            wkv_sb[:, 0:480],
                            start=True, stop=True)
                    # evacuate PSUM -> SBUF (ACT)
                    nc.scalar.copy(
                        kv_sb[:, 6 * g:6 * (g + 1), :].rearrange(
                            "p (c j) m -> p c j m", c=2),
                        kvp[:, :, 0:480].rearrange(
                            "p c (j m) -> p c j m", m=160))

                key4 = kv_sb[:, :, 0:HK].rearrange("p s (h k) -> p s h k", h=H)
                val3 = kv_sb[:, :, HK:160]
                hid4 = hid_sb.rearrange("p (s h k) -> p s h k", s=TPP, h=H)

                sqk = sq_p.tile([128, TPP, H, DIM], F32, name="sqk")
                nc.scalar.activation(sqk[:], key4, AF.Square)
                sqq = sq_p.tile([128, TPP, H, DIM], F32, name="sqq")
                nc.scalar.activation(sqq[:], hid4, AF.Square)

                if apply_g12:
                    prod_in1 = prod_p.tile([128, TPP, H, DIM], F32, name="hidg")
                    nc.vector.tensor_tensor(
                        prod_in1[:], hid4,
                        g12_sb[:].rearrange("p (o h k) -> p o h k", o=1, h=H)
                        .broadcast_to([128, TPP, H, DIM]),
                        op=ALU.mult)
                    prod_in1 = prod_in1[:]
                else:
                    prod_in1 = hid4

                prod = prod_p.tile([128, TPP, H, DIM], F32, name="prod")
                nc.gpsimd.tensor_tensor(prod[:], key4, prod_in1, op=ALU.mult)

                nc.vector.reduce_sum(msk_st[:, bb, :, :], sqk[:], axis=AX.X)
                nc.vector.reduce_sum(msq_st[:, bb, :, :], sqq[:], axis=AX.X)
                nc.vector.reduce_sum(dot_st[:, bb, :, :], prod[:], axis=AX.X)
                nc.scalar.copy(val_st[:, bb, :, :], val3)

            # ---- superblock scalar tail ----
            FT = SB_BLKS * TPP * H
            sk = tail_p.tile([128, FT], F32, name="sk")
            msk_f = msk_st[:].rearrange("p a b c -> p (a b c)")
            msq_f = msq_st[:].rearrange("p a b c -> p (a b c)")
            dot_f = dot_st[:].rearrange("p a b c -> p (a b c)")
            # sk = sqrt(sum_k key^2 + 32*eps) = sqrt(32)*sqrt(mean+eps)
            nc.scalar.activation(sk[:], msk_f, AF.Sqrt, bias=eps_k[:])
            sq2 = tail_p.tile([128, FT], F32, name="sq2")
            nc.scalar.activation(sq2[:], msq_f, AF.Sqrt,
                                 bias=eps_q[:], scale=1.0 / 32.0)
            sg = tail_p.tile([128, FT], F32, name="sg")
            nc.scalar.activation(sg[:], dot_f, AF.Sign)
            den = tail_p.tile([128, FT], F32, name="den")
            nc.vector.tensor_tensor(den[:], sk[:], sq2[:], op=ALU.mult)
            rden = tail_p.tile([128, FT], F32, name="rden")
            nc.vector.reciprocal(rden[:], den[:])
            g0 = tail_p.tile([128, FT], F32, name="g0")
            nc.vector.tensor_tensor(g0[:], dot_f, rden[:], op=ALU.mult)
            aa = tail_p.tile([128, FT], F32, name="aa")
            nc.scalar.activation(aa[:], g0[:], AF.Abs)
            m = tail_p.tile([128, FT], F32, name="m")
            nc.vector.tensor_scalar(m[:], aa[:], 1e-6, None, op0=ALU.max)
            r = tail_p.tile([128, FT], F32, name="r")
            nc.scalar.activation(r[:], m[:], AF.Sqrt)
            rs = tail_p.tile([128, FT], F32, name="rs")
            nc.vector.tensor_tensor(rs[:], r[:], sg[:], op=ALU.mult)
            gate = tail_p.tile([128, SB_BLKS, TPP, H], F32, name="gate")
            nc.scalar.activation(
                gate[:].rearrange("p a b c -> p (a b c)"), rs[:], AF.Sigmoid)

            # ---- final gating + store ----
            for bb in range(SB_BLKS):
                b = sb * SB_BLKS + bb
                t0 = T0S[b]
                out_sb = out_p.tile([128, TPP, H, DIM], F32, name="out_sb")
                gate_b = gate[:, bb, :, :].unsqueeze(3)
                val_b = val_st[:, bb, :, :].unsqueeze(2)
                # split final elementwise mul between DVE (3/18) and GPSIMD
                SPL = 3
                nc.vector.tensor_tensor(
                    out_sb[:, 0:SPL, :, :],
                    gate_b[:, 0:SPL, :, :].broadcast_to([128, SPL, H, DIM]),
                    val_b[:, 0:SPL, :, :].broadcast_to([128, SPL, H, DIM]),
                    op=ALU.mult)
                nc.gpsimd.tensor_tensor(
                    out_sb[:, SPL:TPP, :, :],
                    gate_b[:, SPL:TPP, :, :].broadcast_to(
                        [128, TPP - SPL, H, DIM]),
                    val_b[:, SPL:TPP, :, :].broadcast_to(
                        [128, TPP - SPL, H, DIM]),
                    op=ALU.mult)
                nc.sync.dma_start(
                    out_d[t0 * HK:(t0 + BLK) * HK].rearrange(
                        "(p f) -> p f", p=128),
                    out_sb[:].rearrange("p a b c -> p (a b c)"))

    nc.compile()
    return nc


def _prep_consts(Wv, bv, Wk, bk):
    # Wkv_cat[d, h*32+k] = Wk[h,k,d];  Wkv_cat[d, 128+v] = Wv[v,d]
    wkv_cat = np.zeros((DIM, 160), dtype=np.float32)
    wkv_cat[:, 0:HK] = np.transpose(Wk, (2, 0, 1)).reshape(DIM, HK)
    wkv_cat[:, HK:160] = Wv.T
    bias_cat = np.concatenate(
        [bk.reshape(HK).astype(np.float32), bv.astype(np.float32)])
    wkv = np.zeros((128, 480), dtype=np.float32)
    for j in range(3):
        wkv[32 * j:32 * (j + 1), 160 * j:160 * (j + 1)] = wkv_cat
    wkv[96, :] = np.tile(bias_cat, 3)
    ident = np.eye(128, dtype=np.float32)
    return wkv, ident


_CACHE = {}


def kernel_with_results(embeddings, hidden_states, Wv, bv, Wk, bk, g1, g2,
                        **run_kwargs):
    embeddings = np.ascontiguousarray(np.asarray(embeddings, dtype=np.float32))
    hidden_states = np.ascontiguousarray(
        np.asarray(hidden_states, dtype=np.float32))
    Wv = np.asarray(Wv, dtype=np.float32)
    bv = np.asarray(bv, dtype=np.float32)
    Wk = np.asarray(Wk, dtype=np.float32)
    bk = np.asarray(bk, dtype=np.float32)
    g12 = (np.asarray(g1, np.float32) * np.asarray(g2, np.float32))
    apply_g12 = not np.all(g12 == 1.0)

    if apply_g12 not in _CACHE:
        _CACHE[apply_g12] = _build_nc(apply_g12)
    nc = _CACHE[apply_g12]

    wkv, ident = _prep_consts(Wv, bv, Wk, bk)

    emb_flat = embeddings.reshape(TOK, DIM)
    hid_flat = hidden_states.reshape(TOK, HK)

    in_maps = []
    for c in range(NCORES):
        m = {
            "emb": np.ascontiguousarray(
                emb_flat[c * TPC:(c + 1) * TPC]).reshape(-1),
            "hid": np.ascontiguousarray(
                hid_flat[c * TPC:(c + 1) * TPC]).reshape(-1),
            "wkv": wkv,
            "ident": ident,
        }
        if apply_g12:
            m["g12"] = np.tile(
                g12.reshape(1, HK), (128, 1)).astype(np.float32)
        in_maps.append(m)

    res = run_bass_kernel_spmd(nc, in_maps, core_ids=list(range(NCORES)),
                               **run_kwargs)
    out = np.concatenate(
        [res.results[c]["out"].reshape(TPC, HK) for c in range(NCORES)],
        axis=0)
    return out.reshape(B, S, H, DIM), res


def kernel(embeddings, hidden_states, Wv, bv, Wk, bk, g1, g2):
    out, _ = kernel_with_results(
        embeddings, hidden_states, Wv, bv, Wk, bk, g1, g2)
    return out
